# revision 38
# baseline (speedup 1.0000x reference)
"""GCLConv (GNN message passing) Trainium2 kernel — 8-core edge-parallel.

v3 (supertiled, edge-major layer 2, fused DVE ops, paired row gathers):
  - Host: shard by destination node range (6272 nodes/core, no cross-core
    reduction); edges grouped by (col-half, dest-window), per-group padding
    max-reduced across cores so one SPMD program serves all 8 cores.
  - Row gathers use two streams per group: 512B pair descriptors (rows
    2a/2a+1 serving an even-row + an odd-row edge, dodging the sub-512B DMA
    penalty) + 256B single descriptors; npairs min-reduced across cores.
  - Edge phase per supertile (<=8 subtiles, single group):
      ps1[H,S*128] = eW1topT@rows + eW1botT@cols; m1 = silu(ps1+eb1) (1 ACT)
      layer 2 EDGE-major: ps2em[P,*] = ones x eb2 + m1_jT @ eW2 per subtile;
      m2em = silu(ps2em) (1 ACT per 4 subtiles, PSUM->SBUF move included)
      att_pre = sum(m2em*aW) per edge (DVE mult+segmented reduce)
      att = 0.5*tanh(0.5*att_pre+0.5*ab)+0.5 (1 ACT [P,S] + 1 DVE affine)
      S'_j = (jconst == rel_j)*att_j  (ONE fused 2-scalar DVE tensor_scalar)
      aggT_w[H,WIN] += matmul(lhsT=m2em_j, rhs=S'_j): feature-major
      aggregation directly - no transposes anywhere in the kernel.
  - Node phase interleaved per window during the half-1 sweep; aggT feeds
    nW1 matmul directly; residual add on DVE.
  - sigmoid(x)=0.5*tanh(x/2)+0.5 so Silu/Tanh/Copy share one ACT table.
  - Software-pipelined emission + deep tile pools for cross-supertile
    overlap; gathers prefetched 3 batches ahead.
  HW notes: GPSIMD must not touch PSUM; matmul free dim <= 512;
  tensor_tensor_reduce crashes HW (replaced by mult+reduce).
"""
import os
import sys

sys.path.insert(0, "/opt/trn_rl_repo")

import numpy as np
import ml_dtypes

import concourse.bass as bass
import concourse.bacc as bacc
import concourse.mybir as mybir
import concourse.tile as tile
from concourse import bass_utils

BF16 = ml_dtypes.bfloat16

N = 50000
E = 800000
D = 128
H = 128
P = 128
NCORES = 8
WIN = 128                  # nodes per aggregation window
NW = 49                    # windows per core
SHARD = WIN * NW           # 6272 nodes per core
NPAD = SHARD * NCORES      # 50176
COL_SPLIT = 25088          # col gather table split (both halves < 32768)
COL_HI = NPAD - COL_SPLIT  # 25088
GB = 32                    # subtiles per gather batch
SUP = 8                    # stage-A supertile width (subtiles)
SUB = 4                    # stage-B chunk width
NORM = 100.0

FP32 = mybir.dt.float32
BF = mybir.dt.bfloat16
I16 = mybir.dt.int16


def _idx_layout(idx_flat: np.ndarray) -> np.ndarray:
    """Pack int16 indices into the SWDGE layout [128, n/16]:
    index i -> partition i%16, col i//16, replicated across 8 groups."""
    n = idx_flat.shape[0]
    assert n % 16 == 0
    arr = idx_flat.reshape(n // 16, 16).T.astype(np.int16)  # [16, n/16]
    return np.tile(arr, (8, 1))                             # [128, n/16]


def _preprocess(h: np.ndarray, edge_index: np.ndarray):
    """Build per-core edge tiles + metadata. Returns host data dict.

    Row gathers are split into two streams per group:
      - pair stream: 512B descriptors from the pair table (rows 2a, 2a+1),
        each serving one even-row edge (j=0 slot) + one odd-row edge (j=1).
      - single stream: 256B descriptors (leftover edges + pads).
    npairs per group is min-reduced across cores so the SPMD program is
    uniform; excess pairs on other cores are demoted to singles.
    Slot order within a group: [pair-firsts | pair-seconds | singles|pads].
    """
    row = np.asarray(edge_index[0], dtype=np.int64)
    col = np.asarray(edge_index[1], dtype=np.int64)

    core_of = row // SHARD
    half_of = (col >= COL_SPLIT).astype(np.int64)

    # group counts per (core, half, window)
    win_of = (row % SHARD) // WIN
    counts = np.zeros((NCORES, 2, NW), dtype=np.int64)
    np.add.at(counts, (core_of, half_of, win_of), 1)
    tiles_per_group = np.maximum(1, -(-counts // P))        # ceil, min 1
    T_hw = tiles_per_group.max(axis=0)                      # [2, NW] uniform
    NT = int(T_hw.sum())

    # per-core (half, window) edge lists, sorted by row within group
    per_core = []  # [k][hf][w] -> (rows, cols) row-sorted, cols table-local
    formable = np.zeros((NCORES, 2, NW), dtype=np.int64)
    for k in range(NCORES):
        m = core_of == k
        rk, ck, hk, wk = row[m] - k * SHARD, col[m], half_of[m], win_of[m]
        order = np.lexsort((rk, wk, hk))
        rk, ck, hk, wk = rk[order], ck[order], hk[order], wk[order]
        cnt = np.zeros((2, NW), dtype=np.int64)
        np.add.at(cnt, (hk, wk), 1)
        lists = [[None] * NW, [None] * NW]
        pos = 0
        for hf in range(2):
            for w in range(NW):
                c = int(cnt[hf, w])
                r_g = rk[pos:pos + c]
                c_g = ck[pos:pos + c] - hf * COL_SPLIT
                pos += c
                lists[hf][w] = (r_g, c_g)
                blk = r_g >> 1
                ne = np.bincount(blk[(r_g & 1) == 0], minlength=SHARD // 2)
                no = np.bincount(blk[(r_g & 1) == 1], minlength=SHARD // 2)
                formable[k, hf, w] = np.minimum(ne, no).sum()
        per_core.append(lists)
    npairs = formable.min(axis=0)                           # [2, NW] uniform
    if os.environ.get('NOPAIR'):
        npairs = npairs * 0

    # per-core slot layouts + gather streams
    col_idx = np.empty((NCORES, 128, NT * 8), dtype=np.int16)
    rel_row = np.empty((NCORES, 128, NT), dtype=np.float32)
    pair_streams = [[] for _ in range(NCORES)]   # per group arrays
    sing_streams = [[] for _ in range(NCORES)]
    for k in range(NCORES):
        cols_l, rel_l = [], []
        for hf in range(2):
            for w in range(NW):
                r_g, c_g = per_core[k][hf][w]
                c = r_g.shape[0]
                npg = int(npairs[hf, w])
                slots_g = int(T_hw[hf, w]) * P
                # match pairs per row-block: even-row edge + odd-row edge
                blk = r_g >> 1
                par = r_g & 1
                # indices of edges grouped by (block, parity); r_g sorted
                first_i, second_i = [], []
                i = 0
                n_found = 0
                while i < c and n_found < npg:
                    b = blk[i]
                    j = i
                    while j < c and blk[j] == b:
                        j += 1
                    ev = [x for x in range(i, j) if par[x] == 0]
                    od = [x for x in range(i, j) if par[x] == 1]
                    take = min(len(ev), len(od), npg - n_found)
                    first_i += ev[:take]
                    second_i += od[:take]
                    n_found += take
                    i = j
                assert n_found == npg, (k, hf, w, n_found, npg)
                used = np.zeros(c, dtype=bool)
                used[first_i] = True
                used[second_i] = True
                sing_i = np.nonzero(~used)[0]
                n_sing_slots = slots_g - 2 * npg
                n_pad = n_sing_slots - sing_i.shape[0]
                assert n_pad >= 0
                # slot order: firsts | seconds | singles | pads
                perm = np.concatenate([
                    np.asarray(first_i, np.int64),
                    np.asarray(second_i, np.int64), sing_i])
                c_s = np.concatenate([c_g[perm], np.zeros(n_pad, np.int64)])
                rel_s = np.concatenate([
                    (r_g[perm] % WIN).astype(np.float32),
                    np.full(n_pad, 255.0, np.float32)])
                cols_l.append(c_s)
                rel_l.append(rel_s)
                pair_streams[k].append(
                    (blk[first_i] if npg else np.zeros(0, np.int64)))
                sing_streams[k].append(np.concatenate(
                    [r_g[sing_i], np.zeros(n_pad, np.int64)]))
        c_all = np.concatenate(cols_l)
        rel_all = np.concatenate(rel_l)
        assert c_all.shape[0] == NT * P
        col_idx[k] = _idx_layout(c_all.astype(np.int16))
        rel_row[k] = rel_all.reshape(NT, P).T.astype(np.float32)

    # group-aligned batches (never crossing the half boundary)
    groups_meta = []  # (g0_tile, n_t, hf, w)
    t_i = 0
    for hf in range(2):
        for w in range(NW):
            groups_meta.append((t_i, int(T_hw[hf, w]), hf, w))
            t_i += int(T_hw[hf, w])
    batches_meta = []  # dict(gis, t0, n_t, pd, sd)
    gi = 0
    while gi < len(groups_meta):
        hf0 = groups_meta[gi][2]
        gis = [gi]
        n_t = groups_meta[gi][1]
        gi += 1
        while (gi < len(groups_meta) and groups_meta[gi][2] == hf0
               and n_t + groups_meta[gi][1] <= GB):
            gis.append(gi)
            n_t += groups_meta[gi][1]
            gi += 1
        pd_raw = sum(int(npairs.ravel()[g]) for g in gis)
        sd_raw = sum(int(T_hw.ravel()[g]) * P - 2 * int(npairs.ravel()[g])
                     for g in gis)
        batches_meta.append(dict(
            gis=gis, t0=groups_meta[gis[0]][0], n_t=n_t,
            pd=-(-pd_raw // 128) * 128, pd_raw=pd_raw,
            sd=-(-sd_raw // 128) * 128, sd_raw=sd_raw))

    # concatenated per-batch 128-padded idx streams
    pair_idx = [[] for _ in range(NCORES)]
    sing_idx = [[] for _ in range(NCORES)]
    for k in range(NCORES):
        for bm in batches_meta:
            pstr = np.concatenate(
                [pair_streams[k][g] for g in bm["gis"]]
                + [np.zeros(bm["pd"] - bm["pd_raw"], np.int64)])
            sstr = np.concatenate(
                [sing_streams[k][g] for g in bm["gis"]]
                + [np.zeros(bm["sd"] - bm["sd_raw"], np.int64)])
            pair_idx[k].append(_idx_layout(pstr.astype(np.int16)))
            sing_idx[k].append(_idx_layout(sstr.astype(np.int16)))
    pair_idx = [np.hstack(x) if x and sum(a.shape[1] for a in x) else
                np.zeros((128, 8), np.int16) for x in pair_idx]
    sing_idx = [np.hstack(x) if x and sum(a.shape[1] for a in x) else
                np.zeros((128, 8), np.int16) for x in sing_idx]

    # gather tables
    h_pad = np.zeros((NPAD, D), dtype=np.float32)
    h_pad[:N] = h
    h_bf = h_pad.astype(BF16)
    hA = np.ascontiguousarray(h_bf[:COL_SPLIT])
    hB = np.ascontiguousarray(h_bf[COL_SPLIT:])
    hrow = h_bf.reshape(NCORES, SHARD, D)                   # per-core shard

    # node-phase buffers per core
    hsh = h_pad.reshape(NCORES, NW, WIN, D)
    h_own = np.ascontiguousarray(
        hsh.transpose(0, 2, 1, 3).reshape(NCORES, WIN, NW * D))  # [128, w*128+d]
    hT = np.ascontiguousarray(
        hsh.transpose(0, 3, 1, 2).reshape(NCORES, D, NW * WIN)).astype(BF16)

    return dict(NT=NT, T_hw=T_hw, npairs=npairs, groups_meta=groups_meta,
                batches_meta=batches_meta, col_idx=col_idx,
                pair_idx=pair_idx, sing_idx=sing_idx,
                rel_row=rel_row, hA=hA, hB=hB, hrow=hrow,
                h_own=h_own, hT=hT)


def _build(nc: bass.Bass, prep, act_silu, act_tanh):
    """Emit the SPMD program from uniform metadata in prep."""
    NT = prep["NT"]
    T_hw = prep["T_hw"]
    npairs = prep["npairs"]
    groups_meta = prep["groups_meta"]
    batches_meta = prep["batches_meta"]
    PCOLS = prep["pair_idx"][0].shape[1]
    SCOLS = prep["sing_idx"][0].shape[1]

    dt = nc.dram_tensor
    hA_t = dt("hA", [COL_SPLIT, D], BF, kind="ExternalInput")
    hB_t = dt("hB", [COL_HI, D], BF, kind="ExternalInput")
    hrow_t = dt("hrow", [SHARD, D], BF, kind="ExternalInput")
    hrowp_t = dt("hrowp", [SHARD // 2, 2 * D], BF, kind="ExternalInput")
    cidx_t = dt("col_idx", [128, NT * 8], I16, kind="ExternalInput")
    pidx_t = dt("pair_idx", [128, PCOLS], I16, kind="ExternalInput")
    sidx_t = dt("sing_idx", [128, SCOLS], I16, kind="ExternalInput")
    rel_t = dt("rel_row", [128, NT], FP32, kind="ExternalInput")
    hown_t = dt("h_own", [WIN, NW * D], FP32, kind="ExternalInput")
    hT_t = dt("hT", [D, NW * WIN], BF, kind="ExternalInput")
    # weights / consts (replicated)
    eW1t_t = dt("eW1top", [D, H], BF, kind="ExternalInput")
    eW1b_t = dt("eW1bot", [D, H], BF, kind="ExternalInput")
    eW2_t = dt("eW2", [H, H], BF, kind="ExternalInput")
    eb2r_t = dt("eb2_row4", [1, SUP * H], BF, kind="ExternalInput")
    aWbc_t = dt("aW_bc", [P, H], BF, kind="ExternalInput")
    nW1t_t = dt("nW1top", [D, H], BF, kind="ExternalInput")
    nW1b_t = dt("nW1bot", [H, H], BF, kind="ExternalInput")      # / NORM
    nW2_t = dt("nW2", [H, D], BF, kind="ExternalInput")
    ones_t = dt("ones_row", [1, P], BF, kind="ExternalInput")
    nb2_t = dt("nb2_row", [1, D], BF, kind="ExternalInput")
    eb1_t = dt("eb1", [H, 1], FP32, kind="ExternalInput")
    nb1_t = dt("nb1", [H, 1], FP32, kind="ExternalInput")
    jconst_t = dt("jconst", [P, WIN], BF, kind="ExternalInput")
    ab_t = dt("ab_c", [P, 1], FP32, kind="ExternalInput")  # 0.5*ab
    zero_t = dt("zero_c", [P, 1], FP32, kind="ExternalInput")

    out_t = dt("out", [SHARD, D], FP32, kind="ExternalOutput")

    with tile.TileContext(nc) as tc:
        with (
            tc.tile_pool(name="const", bufs=1) as cp,
            tc.tile_pool(name="gather", bufs=4) as gp,
            tc.tile_pool(name="work", bufs=5) as wp,
            tc.tile_pool(name="agg", bufs=1) as ap_,
            tc.tile_pool(name="ps1p", bufs=2, space="PSUM") as ps1p,
            tc.tile_pool(name="ps2p", bufs=2, space="PSUM") as ps2p,
            tc.tile_pool(name="psagg", bufs=1, space="PSUM") as psg,
            tc.tile_pool(name="psnode", bufs=1, space="PSUM") as psn,
        ):
            # --- resident uploads ---
            def up(shape, dtype, src, tag):
                t = cp.tile(shape, dtype, tag=tag)
                nc.sync.dma_start(out=t[:], in_=src[:])
                return t

            cidx = up([128, NT * 8], I16, cidx_t, "cidx")
            pidx = up([128, PCOLS], I16, pidx_t, "pidx")
            sidx = up([128, SCOLS], I16, sidx_t, "sidx")
            rel = up([128, NT], FP32, rel_t, "rel")
            eW1t = up([D, H], BF, eW1t_t, "eW1t")
            eW1b = up([D, H], BF, eW1b_t, "eW1b")
            eW2 = up([H, H], BF, eW2_t, "eW2")
            eb2r = up([1, SUP * H], BF, eb2r_t, "eb2r")
            aWbc = up([P, H], BF, aWbc_t, "aWbc")
            nW1t = up([D, H], BF, nW1t_t, "nW1t")
            nW1b = up([H, H], BF, nW1b_t, "nW1b")
            nW2 = up([H, D], BF, nW2_t, "nW2")
            ones_r = up([1, P], BF, ones_t, "ones")
            nb2_r = up([1, D], BF, nb2_t, "nb2")
            eb1 = up([H, 1], FP32, eb1_t, "eb1")
            nb1 = up([H, 1], FP32, nb1_t, "nb1")
            jconst = up([P, WIN], BF, jconst_t, "jconst")
            ab_c = up([P, 1], FP32, ab_t, "ab")
            zero_c = up([P, 1], FP32, zero_t, "zero")
            h_own = up([WIN, NW * D], FP32, hown_t, "hown")
            hT = up([D, NW * WIN], BF, hT_t, "hT")

            aggTs = ap_.tile([H, NW * WIN], BF)  # staged feature-major agg
            paggbank = psg.tile([H, 2 * WIN], FP32, space="PSUM", tag="pagg")

            # --- schedule from uniform metadata ---
            # batch bookkeeping: cumulative idx-column bases + buffer sizes
            MAXSL = max(bm["n_t"] for bm in batches_meta) * P
            MAXPD = max(bm["pd"] for bm in batches_meta)
            MAXSD = max(bm["sd"] for bm in batches_meta)
            cum_p = cum_s = 0
            for bm in batches_meta:
                bm["pcol0"] = cum_p // 16
                bm["scol0"] = cum_s // 16
                cum_p += bm["pd"]
                cum_s += bm["sd"]
                # per-group bases within batch (pair descs / single slots)
                pb = sb = 0
                bm["gbase"] = {}
                for g in bm["gis"]:
                    npg = int(npairs.ravel()[g])
                    slg = int(T_hw.ravel()[g]) * P
                    bm["gbase"][g] = (pb, sb)
                    pb += npg
                    sb += slg - 2 * npg
            batch_of_group = {}
            for bi, bm in enumerate(batches_meta):
                for g in bm["gis"]:
                    batch_of_group[g] = bi

            gtiles = {}

            def emit_batch(bi):
                bm = batches_meta[bi]
                hf0 = groups_meta[bm["gis"][0]][2]
                src = hA_t if hf0 == 0 else hB_t
                nsl_b = bm["n_t"] * P
                ct = gp.tile([128, MAXSL], BF, tag="gcol")
                pt = st = None
                if bm["pd"]:
                    pt = gp.tile([128, 2 * MAXPD], BF, tag="gpair")
                if bm["sd"]:
                    st = gp.tile([128, MAXSD], BF, tag="gsing")
                nc.gpsimd.dma_gather(
                    out_ap=ct[:, :nsl_b].rearrange("p (a n) -> p a n", a=1),
                    in_ap=src[:],
                    idxs_ap=cidx[:, bm["t0"] * 8:(bm["t0"] + bm["n_t"]) * 8],
                    num_idxs=nsl_b, num_idxs_reg=nsl_b,
                    elem_size=D, transpose=True, single_packet=False)
                if bm["pd"]:
                    nc.gpsimd.dma_gather(
                        out_ap=pt[:, :2 * bm["pd"]].rearrange(
                            "p (a n) -> p a n", a=2),
                        in_ap=hrowp_t[:],
                        idxs_ap=pidx[:, bm["pcol0"]:
                                     bm["pcol0"] + bm["pd"] // 16],
                        num_idxs=bm["pd"], num_idxs_reg=bm["pd"],
                        elem_size=2 * D, transpose=True, single_packet=False)
                if bm["sd"]:
                    nc.gpsimd.dma_gather(
                        out_ap=st[:, :bm["sd"]].rearrange(
                            "p (a n) -> p a n", a=1),
                        in_ap=hrow_t[:],
                        idxs_ap=sidx[:, bm["scol0"]:
                                     bm["scol0"] + bm["sd"] // 16],
                        num_idxs=bm["sd"], num_idxs_reg=bm["sd"],
                        elem_size=D, transpose=True, single_packet=False)
                gtiles[bi] = (ct, pt, st)

            # supertile schedule: chunks of <=SUP subtiles within a group
            sts = []  # (t, S, gi, g0, g_end, hf, w)
            for gi, (g0, n_t, hf, w) in enumerate(groups_meta):
                t = g0
                g_end = g0 + n_t
                while t < g_end:
                    S = min(SUP, g_end - t)
                    sts.append((t, S, gi, g0, g_end, hf, w))
                    t += S

            state = {}

            def stage_a(n):
                """gather prefetch + layer-1 matmuls + silu1"""
                t, S, gi, g0, g_end, hf, w = sts[n]
                bi = batch_of_group[gi]
                bm = batches_meta[bi]
                if t == g0 and gi == bm["gis"][0]:
                    for nb in (bi + 1, bi + 2, bi + 3):
                        if nb < len(batches_meta) and nb not in gtiles:
                            emit_batch(nb)
                ct, pt, st = gtiles[bi]
                nsl = S * P
                npg = int(npairs.ravel()[gi])
                slg = (g_end - g0) * P
                pb, sb = bm["gbase"][gi]
                ps1 = ps1p.tile([H, SUP * P], FP32, space="PSUM", tag="ps1")
                # row contribution: intersect supertile slots with the 3 runs
                a0 = (t - g0) * P           # group-local slot range
                a1 = a0 + nsl
                runs = (  # (run start, run end, buf, buf base)
                    (0, npg, pt, pb),
                    (npg, 2 * npg, pt, bm["pd"] + pb - npg),
                    (2 * npg, slg, st, sb - 2 * npg),
                )
                first = True
                for (r0, r1, buf, bb) in runs:
                    x0, x1 = max(a0, r0), min(a1, r1)
                    while x0 < x1:
                        xm = min(x0 + 512, x1)
                        nc.tensor.matmul(ps1[:, x0 - a0:xm - a0],
                                         lhsT=eW1t[:],
                                         rhs=buf[:, bb + x0:bb + xm],
                                         start=True, stop=False)
                        x0 = xm
                        first = False
                assert not first
                # col contribution (contiguous per batch)
                e0 = (t - bm["t0"]) * P
                for c0 in range(0, nsl, 512):
                    c1 = min(c0 + 512, nsl)
                    nc.tensor.matmul(ps1[:, c0:c1], lhsT=eW1b[:],
                                     rhs=ct[:, e0 + c0:e0 + c1],
                                     start=False, stop=True)
                m1 = wp.tile([H, SUP * P], BF, tag="m1")
                nc.scalar.activation(m1[:, :nsl], ps1[:, :nsl], act_silu,
                                     bias=eb1[:])
                state[n] = (m1, None, None)

            def stage_b_chunk(n, c):
                """layer 2 + silu2 + att reduce for subtiles [c, c+SUB)"""
                t, S, gi, g0, g_end, hf, w = sts[n]
                m1 = state[n][0]
                C = min(SUB, S - c)
                if c == 0:
                    attp_new = wp.tile([P, SUP], FP32, tag="attp")
                    state[n] = (m1, attp_new, [])
                attp = state[n][1]
                ps2 = ps2p.tile([P, SUB * H], FP32, space="PSUM", tag="ps2")
                nc.tensor.matmul(ps2[:, :C * H], lhsT=ones_r[:],
                                 rhs=eb2r[:, :C * H], start=True, stop=False)
                for j in range(C):
                    nc.tensor.matmul(
                        ps2[:, j * H:(j + 1) * H],
                        lhsT=m1[:, (c + j) * P:(c + j + 1) * P],
                        rhs=eW2[:], start=False, stop=(j == C - 1),
                        skip_group_check=True)
                m2em = wp.tile([P, SUB * H], BF, tag="m2em")
                nc.scalar.activation(m2em[:, :C * H], ps2[:, :C * H],
                                     act_silu, bias=zero_c[:])
                state[n][2].append(m2em)

                scr = wp.tile([P, H], BF, tag="scr")
                if not os.environ.get("FUSED_TTR"):
                    scrf = wp.tile([P, SUB * H], BF, tag="scrf")
                    for j in range(C):
                        nc.vector.tensor_tensor(
                            out=scrf[:, j * H:(j + 1) * H],
                            in0=m2em[:, j * H:(j + 1) * H],
                            in1=aWbc[:], op=mybir.AluOpType.mult)
                    nc.vector.tensor_reduce(
                        out=attp[:, c:c + C],
                        in_=scrf[:, :C * H].rearrange("p (c h) -> p c h", c=C),
                        axis=mybir.AxisListType.X, op=mybir.AluOpType.add)
                else:
                    for j in range(C):
                        nc.vector.tensor_tensor_reduce(
                            out=scr[:], in0=m2em[:, j * H:(j + 1) * H],
                            in1=aWbc[:], scale=1.0, scalar=0.0,
                            op0=mybir.AluOpType.mult, op1=mybir.AluOpType.add,
                            accum_out=attp[:, c + j:c + j + 1])

            def stage_b_att(n):
                """supertile-level attention + scatter"""
                t, S, gi, g0, g_end, hf, w = sts[n]
                m1, attp, m2ems = state[n]
                pagg_cur = paggbank[:, (gi % 2) * WIN:(gi % 2 + 1) * WIN]
                att_t = wp.tile([P, SUP], FP32, tag="att_t")
                nc.scalar.activation(att_t[:, :S], attp[:, :S], act_tanh,
                                     bias=ab_c[:], scale=0.5)
                att = wp.tile([P, SUP], FP32, tag="att")
                nc.vector.tensor_scalar(
                    out=att[:, :S], in0=att_t[:, :S], scalar1=0.5,
                    scalar2=0.5, op0=mybir.AluOpType.mult,
                    op1=mybir.AluOpType.add)
                for j in range(S):
                    tj = t + j
                    m2em = m2ems[j // SUB]
                    jj = j % SUB
                    sp = wp.tile([P, WIN], BF, tag="sp")
                    if False:
                        pass
                    else:
                        nc.vector.tensor_scalar(
                            out=sp[:], in0=jconst[:],
                            scalar1=rel[:, tj:tj + 1],
                            scalar2=att[:, j:j + 1],
                            op0=mybir.AluOpType.is_equal,
                            op1=mybir.AluOpType.mult)
                    nc.tensor.matmul(pagg_cur[:],
                                     lhsT=m2em[:, jj * H:(jj + 1) * H],
                                     rhs=sp[:],
                                     start=(tj == g0),
                                     stop=(tj == g_end - 1))

            def stage_b_tail(n):
                """group staging + node phase after the scatter"""
                t, S, gi, g0, g_end, hf, w = sts[n]
                stage_b_att(n)
                del state[n]
                wsl = slice(w * WIN, (w + 1) * WIN)
                pagg_cur = paggbank[:, (gi % 2) * WIN:(gi % 2 + 1) * WIN]
                if t + S < g_end:
                    return
                # group end: stage aggregate (Pool engine)
                if hf == 0:
                    nc.vector.tensor_copy(aggTs[:, wsl], pagg_cur[:])
                    return
                nc.vector.tensor_add(out=aggTs[:, wsl], in0=pagg_cur[:],
                                     in1=aggTs[:, wsl])
                # node phase for window w, interleaved with edge work
                psn1 = psn.tile([H, WIN], FP32, space="PSUM", tag="psn1")
                nc.tensor.matmul(psn1[:], lhsT=nW1t[:], rhs=hT[:, wsl],
                                 start=True, stop=False)
                nc.tensor.matmul(psn1[:], lhsT=nW1b[:], rhs=aggTs[:, wsl],
                                 start=False, stop=True)
                y1 = wp.tile([H, WIN], BF, tag="y1")
                nc.scalar.activation(y1[:], psn1[:], act_silu, bias=nb1[:])

                psn2 = psn.tile([WIN, D], FP32, space="PSUM", tag="psn1")
                nc.tensor.matmul(psn2[:], lhsT=y1[:], rhs=nW2[:],
                                 start=True, stop=False)
                nc.tensor.matmul(psn2[:], lhsT=ones_r[:, :WIN],
                                 rhs=nb2_r[:], start=False, stop=True)
                o_sb = wp.tile([WIN, D], FP32, tag="osb")
                nc.vector.tensor_add(out=o_sb[:], in0=psn2[:],
                                     in1=h_own[:, w * D:(w + 1) * D])
                nc.sync.dma_start(out=out_t[w * WIN:(w + 1) * WIN, :],
                                  in_=o_sb[:])

            # pipelined emission: B(n).c0, A(n+1), B(n).c1.., tail(n)
            emit_batch(0)
            emit_batch(1)
            stage_a(0)
            for n in range(len(sts)):
                S = sts[n][1]
                stage_b_chunk(n, 0)
                if n + 1 < len(sts):
                    stage_a(n + 1)
                for c in range(SUB, S, SUB):
                    stage_b_chunk(n, c)
                stage_b_tail(n)
    return nc


def _make_in_maps(prep, inputs):
    eW1 = np.asarray(inputs["eW1"], np.float32)
    aW = np.asarray(inputs["aW"], np.float32).reshape(1, H)
    nW1 = np.asarray(inputs["nW1"], np.float32)
    eb2 = np.asarray(inputs["eb2"], np.float32).reshape(1, H)
    jconst = np.broadcast_to(np.arange(WIN, dtype=np.float32)[None, :], (P, WIN))
    common = {
        "hA": prep["hA"], "hB": prep["hB"],
        "eW1top": eW1[:D].astype(BF16), "eW1bot": eW1[D:].astype(BF16),
        "eW2": np.asarray(inputs["eW2"], np.float32).astype(BF16),
        "eb2_row4": np.ascontiguousarray(
            np.tile(eb2, (1, SUP))).astype(BF16),
        "aW_bc": np.ascontiguousarray(
            np.broadcast_to(aW, (P, H))).astype(BF16),
        "nW1top": nW1[:D].astype(BF16),
        "nW1bot": (nW1[D:] / NORM).astype(BF16),
        "nW2": np.asarray(inputs["nW2"], np.float32).astype(BF16),
        "ones_row": np.ones((1, P), BF16),
        "nb2_row": np.asarray(inputs["nb2"], np.float32).reshape(1, D).astype(BF16),
        "eb1": np.asarray(inputs["eb1"], np.float32).reshape(H, 1),
        "nb1": np.asarray(inputs["nb1"], np.float32).reshape(H, 1),
        "jconst": np.ascontiguousarray(jconst).astype(BF16),
        # tanh form: sigmoid(x+ab) = 0.5*tanh(0.5x + 0.5ab) + 0.5
        "ab_c": np.full((P, 1), 0.5 * float(np.asarray(inputs["ab"]).ravel()[0]),
                        dtype=np.float32),
        "zero_c": np.zeros((P, 1), np.float32),
    }
    in_maps = []
    for k in range(NCORES):
        m = dict(common)
        m["hrow"] = np.ascontiguousarray(prep["hrow"][k])
        m["hrowp"] = np.ascontiguousarray(
            prep["hrow"][k].reshape(SHARD // 2, 2 * D))
        m["col_idx"] = np.ascontiguousarray(prep["col_idx"][k])
        m["pair_idx"] = np.ascontiguousarray(prep["pair_idx"][k])
        m["sing_idx"] = np.ascontiguousarray(prep["sing_idx"][k])
        m["rel_row"] = np.ascontiguousarray(prep["rel_row"][k])
        m["h_own"] = np.ascontiguousarray(prep["h_own"][k])
        m["hT"] = np.ascontiguousarray(prep["hT"][k])
        in_maps.append(m)
    return in_maps


_RUN_KW = {}


def kernel(**inputs) -> np.ndarray:
    h = np.asarray(inputs["h"], np.float32)
    prep = _preprocess(h, np.asarray(inputs["edge_index"]))

    nc = bacc.Bacc("TRN2", target_bir_lowering=False, debug=False,
                   num_devices=NCORES)
    _build(nc, prep,
           act_silu=mybir.ActivationFunctionType.Silu,
           act_tanh=mybir.ActivationFunctionType.Tanh)
    nc.compile()

    in_maps = _make_in_maps(prep, inputs)
    res = bass_utils.run_bass_kernel_spmd(
        nc, in_maps, core_ids=list(range(NCORES)), **_RUN_KW)
    out = np.empty((NPAD, D), dtype=np.float32)
    for k in range(NCORES):
        out[k * SHARD:(k + 1) * SHARD] = np.asarray(res.results[k]["out"])
    kernel._last_results = res
    kernel._last_nc = nc
    return out[:N]
